# revision 5
# baseline (speedup 1.0000x reference)
"""Trainium2 Bass kernel for nn_EnhancedExternalMemoryBank (retrieval_knn).

Sharding: one head per NeuronCore (8 heads, 8 cores). Queries are sliced
per-head on the host (layout only); all arithmetic (chunk-mean, score GEMM,
top-k, chunk gather) runs on-device. Host concatenates per-head outputs.

Per-core pipeline:
  phase 1: chunk-mean of mem_keys (GPSIMD pool_avg) -> PE transpose -> ckT[64,S]
  phase 2: per 128-query tile:
     PE: scores = qT.T @ ckT (fp32) into PSUM, ACT drains PSUM->SBUF
     DVE: max8 on each half row, merge -> top-8 values; find_index8 per half
          (-1-skip semantics make cross-half duplicate handling exact)
     GPSIMD: indirect DMA gathers top-4 chunks (2KB each) of keys/vals
     big contiguous DMA writes gathered rows to DRAM outputs
"""

import sys

sys.path.insert(0, "/opt/trn_rl_repo")

from contextlib import ExitStack

import numpy as np

import concourse.bass as bass
import concourse.tile as tile
from concourse import bacc, mybir
from concourse.bass import IndirectOffsetOnAxis
from concourse.bass_utils import run_bass_kernel_spmd
from concourse.masks import make_identity

F32 = mybir.dt.float32
F32R = mybir.dt.float32r
U16 = mybir.dt.uint16
U32 = mybir.dt.uint32

# Problem geometry (hardcoded per spec)
H, S, C, Dh = 8, 16384, 8, 64
L, B, Dm = 2048, 2, 512
N = L * B  # 4096 queries
KPC = 4  # chunks retrieved per query (k_per_chunk)
K = KPC * C  # 32
CHUNK = C * Dh  # 512 f32 = 2KB per chunk

# 'f32'  : exact fp32 matmul (4 cycles/row on PE)
# 'f32r' : single-pass fp32r matmul (1 cycle/row at N>=256; reduced precision,
#          needs producers rounded to fp32r — walrus verifier enforces)
MM_DTYPE = "f32"


def retrieval_kernel(tc, qT, keys, vals, outk, outv, S_, N_):
    """Emit the per-core program.

    qT   : DRAM [Dh, N_] f32      (queries for this head, transposed)
    keys : DRAM [S_, CHUNK] f32   (mem_keys[h] flattened per chunk)
    vals : DRAM [S_, CHUNK] f32
    outk : DRAM [N_, KPC*CHUNK] f32
    outv : DRAM [N_, KPC*CHUNK] f32
    """
    nc = tc.nc
    HALF = S_ // 2
    PIECE = min(2048, HALF)  # PSUM tile free size (4 banks at 2048)
    NSUB = PIECE // 512  # matmuls per PSUM tile
    NPIECE = HALF // PIECE  # PSUM tiles per half
    MT = N_ // 128  # number of 128-query tiles
    ST = S_ // 128  # number of 128-chunk tiles (phase 1)

    mm_cast = (lambda ap: ap.bitcast(F32R)) if MM_DTYPE == "f32r" else (lambda ap: ap)

    with ExitStack() as ctx:
        const_pool = ctx.enter_context(tc.tile_pool(name="const", bufs=1))
        ident = const_pool.tile([128, 128], F32)
        make_identity(nc, ident[:])
        ckT = const_pool.tile([Dh, S_], F32)  # chunk-mean keys, transposed

        # ---------------- phase 1: ckT = mean_c(keys).T ----------------
        with (
            tc.tile_pool(name="p1k", bufs=3) as kp,
            tc.tile_pool(name="p1c", bufs=3) as cp,
            tc.tile_pool(name="p1ps", bufs=2, space="PSUM") as pp,
        ):
            for st in range(ST):
                kt = kp.tile([128, CHUNK], F32)
                nc.sync.dma_start(kt[:], keys[st * 128 : (st + 1) * 128, :])
                # chunk-sum over c via pairwise adds on GpSimd (ranking is
                # scale-invariant, so sum stands in for the reference mean)
                t1 = cp.tile([128, CHUNK // 2], F32, tag="t1")
                nc.gpsimd.tensor_add(t1[:], kt[:, : CHUNK // 2], kt[:, CHUNK // 2 :])
                t2 = cp.tile([128, CHUNK // 4], F32, tag="t2")
                nc.gpsimd.tensor_add(t2[:], t1[:, : CHUNK // 4], t1[:, CHUNK // 4 :])
                ck = cp.tile([128, Dh], F32, tag="ck")
                nc.gpsimd.tensor_add(ck[:], t2[:, :Dh], t2[:, Dh:])
                ps = pp.tile([Dh, 128], F32)
                nc.tensor.transpose(ps[:], ck[:], ident[:])
                nc.scalar.activation(
                    ckT[:, st * 128 : (st + 1) * 128],
                    ps[:],
                    mybir.ActivationFunctionType.Copy,
                )

        # ---------------- phase 2: scores, top-k, gather ----------------
        with (
            tc.tile_pool(name="qm", bufs=2) as qp,
            tc.tile_pool(name="sc", bufs=3) as sp,
            tc.tile_pool(name="sm", bufs=2) as smp,
            tc.tile_pool(name="g", bufs=1) as gp,
            tc.tile_pool(name="ps2", bufs=2, space="PSUM") as pp2,
        ):
            for m in range(MT):
                qTm = qp.tile([Dh, 128], F32)
                nc.sync.dma_start(qTm[:], qT[:, m * 128 : (m + 1) * 128])

                v16 = smp.tile([128, 16], F32, tag="v16")
                halves = []
                for hb in range(2):
                    sc = sp.tile([128, HALF], F32, tag="sc")
                    for pc in range(NPIECE):
                        ps = pp2.tile([128, PIECE], F32)
                        for j in range(NSUB):
                            s0 = hb * HALF + pc * PIECE + j * 512
                            nc.tensor.matmul(
                                ps[:, j * 512 : (j + 1) * 512],
                                lhsT=mm_cast(qTm[:]),
                                rhs=mm_cast(ckT[:, s0 : s0 + 512]),
                                start=True,
                                stop=True,
                            )
                        nc.scalar.activation(
                            sc[:, pc * PIECE : (pc + 1) * PIECE],
                            ps[:],
                            mybir.ActivationFunctionType.Copy,
                        )
                    # top-8 values of this half
                    nc.vector.max(out=v16[:, hb * 8 : (hb + 1) * 8], in_=sc[:])
                    halves.append(sc)

                # merge: exact top-8 values of the full row
                vtop = smp.tile([128, 8], F32, tag="vtop")
                nc.vector.max(out=vtop[:], in_=v16[:])
                # positions in each half (0xFFFF when not found / consumed)
                iA = smp.tile([128, 8], U16, tag="iA")
                nc.vector.max_index(iA[:], vtop[:], halves[0][:])
                iB = smp.tile([128, 8], U16, tag="iB")
                nc.vector.max_index(iB[:], vtop[:], halves[1][:])
                # merge in fp32 (exact for ints < 2^24):
                #   comb = min(iA, min(iB, HALF) + HALF), clamped to S-1
                fA = smp.tile([128, 8], F32, tag="fA")
                nc.vector.tensor_copy(fA[:], iA[:])
                fB = smp.tile([128, 8], F32, tag="fB")
                nc.vector.tensor_copy(fB[:], iB[:])
                nc.vector.tensor_scalar_min(fB[:], fB[:], float(HALF))
                nc.vector.tensor_scalar_add(fB[:], fB[:], float(HALF))
                nc.vector.tensor_tensor(fA[:], fA[:], fB[:], op=mybir.AluOpType.min)
                nc.vector.tensor_scalar_min(fA[:], fA[:], float(S_ - 1))
                comb = smp.tile([128, 8], U32, tag="comb")
                nc.vector.tensor_copy(comb[:], fA[:])

                # gather top-4 chunks of keys and vals by index (2KB each)
                gk = gp.tile([128, KPC * CHUNK], F32, tag="gk")
                gv = gp.tile([128, KPC * CHUNK], F32, tag="gv")
                for j in range(KPC):
                    nc.gpsimd.indirect_dma_start(
                        out=gk[:, j * CHUNK : (j + 1) * CHUNK],
                        out_offset=None,
                        in_=keys[:, :],
                        in_offset=IndirectOffsetOnAxis(ap=comb[:, j : j + 1], axis=0),
                    )
                    nc.gpsimd.indirect_dma_start(
                        out=gv[:, j * CHUNK : (j + 1) * CHUNK],
                        out_offset=None,
                        in_=vals[:, :],
                        in_offset=IndirectOffsetOnAxis(ap=comb[:, j : j + 1], axis=0),
                    )
                nc.sync.dma_start(outk[m * 128 : (m + 1) * 128, :], gk[:])
                nc.sync.dma_start(outv[m * 128 : (m + 1) * 128, :], gv[:])


def build_nc(S_=S, N_=N, debug=False):
    nc = bacc.Bacc("TRN2", target_bir_lowering=False, debug=debug)
    qT = nc.dram_tensor("qT", [Dh, N_], F32, kind="ExternalInput").ap()
    keys = nc.dram_tensor("keys", [S_, CHUNK], F32, kind="ExternalInput").ap()
    vals = nc.dram_tensor("vals", [S_, CHUNK], F32, kind="ExternalInput").ap()
    outk = nc.dram_tensor("outk", [N_, KPC * CHUNK], F32, kind="ExternalOutput").ap()
    outv = nc.dram_tensor("outv", [N_, KPC * CHUNK], F32, kind="ExternalOutput").ap()
    with tile.TileContext(nc) as tc:
        retrieval_kernel(tc, qT, keys, vals, outk, outv, S_, N_)
    nc.compile()
    return nc


_NC = None
LAST_RESULTS = None  # BassKernelResults of the most recent kernel() call


def kernel(queries, mem_keys, mem_vals):
    global _NC, LAST_RESULTS
    if _NC is None:
        _NC = build_nc()

    q = np.asarray(queries, dtype=np.float32).reshape(N, H, Dh)
    in_maps = []
    for h in range(H):
        in_maps.append(
            {
                "qT": np.ascontiguousarray(q[:, h, :].T),
                "keys": np.ascontiguousarray(
                    np.asarray(mem_keys[h], dtype=np.float32).reshape(S, CHUNK)
                ),
                "vals": np.ascontiguousarray(
                    np.asarray(mem_vals[h], dtype=np.float32).reshape(S, CHUNK)
                ),
            }
        )

    res = run_bass_kernel_spmd(nc=_NC, in_maps=in_maps, core_ids=list(range(H)))
    LAST_RESULTS = res

    ks = np.stack(
        [res.results[h]["outk"].reshape(N, K, Dh) for h in range(H)], axis=1
    ).reshape(N * H, K, Dh)
    vs = np.stack(
        [res.results[h]["outv"].reshape(N, K, Dh) for h in range(H)], axis=1
    ).reshape(N * H, K, Dh)
    return np.stack([ks, vs]).astype(np.float32)


def bench_exec_ns(queries, mem_keys, mem_vals, iters=5):
    """Median wall time of the 8-core device execute with inputs pre-placed
    on-device (proxy for HW exec time; excludes host<->device transfers)."""
    global _NC
    if _NC is None:
        _NC = build_nc()
    nc = _NC

    import time

    import jax
    from jax.sharding import Mesh, NamedSharding, PartitionSpec

    from concourse import bass2jax as b2j
    from concourse.bass2jax import _bass_exec_p, partition_id_tensor
    from jax.experimental.shard_map import shard_map

    b2j.install_neuronx_cc_hook()

    q = np.asarray(queries, dtype=np.float32).reshape(N, H, Dh)
    in_maps = []
    for h in range(H):
        in_maps.append(
            {
                "qT": np.ascontiguousarray(q[:, h, :].T),
                "keys": np.ascontiguousarray(
                    np.asarray(mem_keys[h], dtype=np.float32).reshape(S, CHUNK)
                ),
                "vals": np.ascontiguousarray(
                    np.asarray(mem_vals[h], dtype=np.float32).reshape(S, CHUNK)
                ),
            }
        )

    import concourse.mybir as mybir_

    partition_name = nc.partition_id_tensor.name if nc.partition_id_tensor else None
    in_names, out_names, out_avals, zero_outs = [], [], [], []
    for alloc in nc.m.functions[0].allocations:
        if not isinstance(alloc, mybir_.MemoryLocationSet):
            continue
        name = alloc.memorylocations[0].name
        if alloc.kind == "ExternalInput":
            if name != partition_name:
                in_names.append(name)
        elif alloc.kind == "ExternalOutput":
            out_names.append(name)
            shape = tuple(alloc.tensor_shape)
            dtype = mybir_.dt.np(alloc.dtype)
            out_avals.append(jax.core.ShapedArray(shape, dtype))
            zero_outs.append(np.zeros(shape, dtype))
    n_params = len(in_names)
    n_outs = len(out_avals)
    all_in_names = in_names + out_names + ([partition_name] if partition_name else [])
    donate = tuple(range(n_params, n_params + n_outs))

    def _body(*args):
        operands = list(args)
        if partition_name is not None:
            operands.append(partition_id_tensor())
        return tuple(
            _bass_exec_p.bind(
                *operands,
                out_avals=tuple(out_avals),
                in_names=tuple(all_in_names),
                out_names=tuple(out_names),
                lowering_input_output_aliases=(),
                sim_require_finite=True,
                sim_require_nnan=True,
                nc=nc,
            )
        )

    devices = jax.devices()[:H]
    mesh = Mesh(np.asarray(devices), ("core",))
    spec = NamedSharding(mesh, PartitionSpec("core"))
    sharded = jax.jit(
        shard_map(
            _body,
            mesh=mesh,
            in_specs=(PartitionSpec("core"),) * (n_params + n_outs),
            out_specs=(PartitionSpec("core"),) * n_outs,
            check_rep=False,
        ),
        donate_argnums=donate,
        keep_unused=True,
    )
    concat_in = [
        jax.device_put(
            np.concatenate([np.asarray(in_maps[c][nm]) for c in range(H)], axis=0),
            spec,
        )
        for nm in in_names
    ]
    jax.block_until_ready(concat_in)

    times = []
    for i in range(iters + 1):
        zeros = [
            jax.device_put(np.zeros((H * z.shape[0], *z.shape[1:]), z.dtype), spec)
            for z in zero_outs
        ]
        jax.block_until_ready(zeros)
        t0 = time.perf_counter()
        outs = sharded(*concat_in, *zeros)
        jax.block_until_ready(outs)
        t1 = time.perf_counter()
        if i > 0:  # skip compile/warmup iteration
            times.append(t1 - t0)
        del outs
    times.sort()
    return int(times[len(times) // 2] * 1e9)


# revision 17
# speedup vs baseline: 1.0072x; 1.0072x over previous
"""Trainium2 Bass kernel for nn_EnhancedExternalMemoryBank (retrieval_knn).

Sharding: one head per NeuronCore (8 heads, 8 cores). Queries are sliced
per-head on the host (layout only); all arithmetic (chunk-mean, score GEMM,
top-k, chunk gather) runs on-device. Host concatenates per-head outputs.

Per-core pipeline:
  phase 1: chunk-mean of mem_keys (GPSIMD pool_avg) -> PE transpose -> ckT[64,S]
  phase 2: per 128-query tile:
     PE: scores = qT.T @ ckT (fp32) into PSUM, ACT drains PSUM->SBUF
     DVE: max8 on each half row, merge -> top-8 values; find_index8 per half
          (-1-skip semantics make cross-half duplicate handling exact)
     GPSIMD: indirect DMA gathers top-4 chunks (2KB each) of keys/vals
     big contiguous DMA writes gathered rows to DRAM outputs
"""

import sys

sys.path.insert(0, "/opt/trn_rl_repo")

from contextlib import ExitStack

import numpy as np

import concourse.bass as bass
import concourse.tile as tile
from concourse import bacc, mybir
from concourse.bass import IndirectOffsetOnAxis
from concourse.bass_utils import run_bass_kernel_spmd
from concourse.masks import make_identity

F32 = mybir.dt.float32
F32R = mybir.dt.float32r
U16 = mybir.dt.uint16
U32 = mybir.dt.uint32

# Problem geometry (hardcoded per spec)
H, S, C, Dh = 8, 16384, 8, 64
L, B, Dm = 2048, 2, 512
N = L * B  # 4096 queries
KPC = 4  # chunks retrieved per query (k_per_chunk)
K = KPC * C  # 32
CHUNK = C * Dh  # 512 f32 = 2KB per chunk

# 'f32'  : exact fp32 matmul (4 cycles/row on PE)
# 'f32r' : single-pass fp32r matmul (1 cycle/row at N>=256; reduced precision,
#          needs producers rounded to fp32r — walrus verifier enforces)
MM_DTYPE = "f32"  # 'f32r' measured broken on HW: garbage output on every row


def retrieval_kernel(tc, qT, keys, vals, outk, outv, S_, N_, repeat=1):
    """Emit the per-core program.

    qT   : DRAM [Dh, N_] f32      (queries for this head, transposed)
    keys : DRAM [S_, CHUNK] f32   (mem_keys[h] flattened per chunk)
    vals : DRAM [S_, CHUNK] f32
    outk : DRAM [N_, KPC*CHUNK] f32
    outv : DRAM [N_, KPC*CHUNK] f32
    """
    nc = tc.nc
    HALF = S_ // 2
    PIECE = min(2048, HALF)  # PSUM tile free size (4 banks at 2048)
    NSUB = PIECE // 512  # matmuls per PSUM tile
    NPIECE = HALF // PIECE  # PSUM tiles per half
    MT = N_ // 128  # number of 128-query tiles
    ST = S_ // 128  # number of 128-chunk tiles (phase 1)

    for _rep in range(repeat):
        _emit_once(tc, qT, keys, vals, outk, outv, S_, N_)


def _emit_once(tc, qT, keys, vals, outk, outv, S_, N_):
    nc = tc.nc
    HALF = S_ // 2
    PIECE = min(2048, HALF)
    NSUB = PIECE // 512
    NPIECE = HALF // PIECE
    MT = N_ // 128
    ST = S_ // 128
    MM_DT = F32R if MM_DTYPE == "f32r" else F32
    # ckT is split into PIECE-wide sub-tiles so phase-2 matmuls only depend on
    # the phase-1 slice they actually read (earlier pipeline start)
    CKSPLIT = PIECE
    ST_PER_CK = CKSPLIT // 128

    with ExitStack() as ctx:
        const_pool = ctx.enter_context(tc.tile_pool(name="const", bufs=1))
        ident = const_pool.tile([128, 128], F32)
        make_identity(nc, ident[:])
        # chunk-mean keys, transposed, in matmul dtype
        ckTs = [
            const_pool.tile([Dh, CKSPLIT], MM_DT, tag=f"ckT{i}", name=f"ckT{i}")
            for i in range(S_ // CKSPLIT)
        ]

        # ---------------- phase 1: ckT = mean_c(keys).T ----------------
        with (
            tc.tile_pool(name="p1k", bufs=3) as kp,
            tc.tile_pool(name="p1c", bufs=3) as cp,
            tc.tile_pool(name="p1ps", bufs=2, space="PSUM") as pp,
        ):
            for st in range(ST):
                kt = kp.tile([128, CHUNK], F32)
                nc.sync.dma_start(kt[:], keys[st * 128 : (st + 1) * 128, :])
                # chunk-sum over c via pairwise adds on GpSimd (ranking is
                # scale-invariant, so sum stands in for the reference mean)
                t1 = cp.tile([128, CHUNK // 2], F32, tag="t1")
                nc.gpsimd.tensor_add(t1[:], kt[:, : CHUNK // 2], kt[:, CHUNK // 2 :])
                t2 = cp.tile([128, CHUNK // 4], F32, tag="t2")
                nc.gpsimd.tensor_add(t2[:], t1[:, : CHUNK // 4], t1[:, CHUNK // 4 :])
                ck = cp.tile([128, Dh], F32, tag="ck")
                nc.gpsimd.tensor_add(ck[:], t2[:, :Dh], t2[:, Dh:])
                ps = pp.tile([Dh, 128], F32)
                nc.tensor.transpose(ps[:], ck[:], ident[:])
                dst = ckTs[st // ST_PER_CK]
                o = (st % ST_PER_CK) * 128
                nc.scalar.activation(
                    dst[:, o : o + 128], ps[:], mybir.ActivationFunctionType.Copy
                )

        # ---------------- phase 2: scores, top-k, gather ----------------
        with (
            tc.tile_pool(name="qm", bufs=2) as qp,
            tc.tile_pool(name="sc", bufs=3) as sp,
            tc.tile_pool(name="sm", bufs=2) as smp,
            tc.tile_pool(name="g", bufs=1) as gp,
            tc.tile_pool(name="ps2", bufs=2, space="PSUM") as pp2,
        ):
            for m in range(MT):
                qTm = qp.tile([Dh, 128], MM_DT)
                nc.sync.dma_start(qTm[:], qT[:, m * 128 : (m + 1) * 128])

                v16 = smp.tile([128, 16], F32, tag="v16")
                halves = []
                for hb in range(2):
                    sc = sp.tile([128, HALF], F32, tag="sc")
                    for pc in range(NPIECE):
                        ps = pp2.tile([128, PIECE], F32)
                        for j in range(NSUB):
                            s0 = hb * HALF + pc * PIECE + j * 512
                            ck_sub = ckTs[s0 // CKSPLIT]
                            o = s0 % CKSPLIT
                            nc.tensor.matmul(
                                ps[:, j * 512 : (j + 1) * 512],
                                lhsT=qTm[:],
                                rhs=ck_sub[:, o : o + 512],
                                start=True,
                                stop=True,
                            )
                        nc.scalar.activation(
                            sc[:, pc * PIECE : (pc + 1) * PIECE],
                            ps[:],
                            mybir.ActivationFunctionType.Copy,
                        )
                    # top-8 values of this half
                    nc.vector.max(out=v16[:, hb * 8 : (hb + 1) * 8], in_=sc[:])
                    halves.append(sc)

                # merge: exact top-8 values of the full row
                vtop = smp.tile([128, 8], F32, tag="vtop")
                nc.vector.max(out=vtop[:], in_=v16[:])
                # positions in each half (0xFFFF when not found / consumed)
                iA = smp.tile([128, 8], U16, tag="iA")
                nc.vector.max_index(iA[:], vtop[:], halves[0][:])
                iB = smp.tile([128, 8], U16, tag="iB")
                nc.vector.max_index(iB[:], vtop[:], halves[1][:])
                # merge in fp32 (exact for ints < 2^24):
                #   comb = min(iA, min(iB, HALF) + HALF), clamped to S-1
                fA = smp.tile([128, 8], F32, tag="fA")
                nc.vector.tensor_copy(fA[:], iA[:])
                fB = smp.tile([128, 8], F32, tag="fB")
                nc.vector.tensor_copy(fB[:], iB[:])
                nc.vector.tensor_scalar_min(fB[:], fB[:], float(HALF))
                nc.vector.tensor_scalar_add(fB[:], fB[:], float(HALF))
                nc.vector.tensor_tensor(fA[:], fA[:], fB[:], op=mybir.AluOpType.min)
                nc.vector.tensor_scalar_min(fA[:], fA[:], float(S_ - 1))
                comb = smp.tile([128, 8], U32, tag="comb")
                nc.vector.tensor_copy(comb[:], fA[:])

                # gather top-4 chunks of keys and vals by index (2KB each);
                # one indirect DMA per (tensor, j): offsets [128, 1].
                # NOTE: batching offsets as [128, KPC] passes CoreSim but
                # scrambles chunk placement on HW — keep per-column gathers.
                gk = gp.tile([128, KPC * CHUNK], F32, tag="gk")
                gv = gp.tile([128, KPC * CHUNK], F32, tag="gv")
                for j in range(KPC):
                    nc.gpsimd.indirect_dma_start(
                        out=gk[:, j * CHUNK : (j + 1) * CHUNK],
                        out_offset=None,
                        in_=keys[:, :],
                        in_offset=IndirectOffsetOnAxis(ap=comb[:, j : j + 1], axis=0),
                    )
                    nc.gpsimd.indirect_dma_start(
                        out=gv[:, j * CHUNK : (j + 1) * CHUNK],
                        out_offset=None,
                        in_=vals[:, :],
                        in_offset=IndirectOffsetOnAxis(ap=comb[:, j : j + 1], axis=0),
                    )
                nc.sync.dma_start(outk[m * 128 : (m + 1) * 128, :], gk[:])
                nc.sync.dma_start(outv[m * 128 : (m + 1) * 128, :], gv[:])


def build_nc(S_=S, N_=N, debug=False, repeat=1):
    nc = bacc.Bacc("TRN2", target_bir_lowering=False, debug=debug)
    qt_dt = F32R if MM_DTYPE == "f32r" else F32
    qT = nc.dram_tensor("qT", [Dh, N_], qt_dt, kind="ExternalInput").ap()
    keys = nc.dram_tensor("keys", [S_, CHUNK], F32, kind="ExternalInput").ap()
    vals = nc.dram_tensor("vals", [S_, CHUNK], F32, kind="ExternalInput").ap()
    outk = nc.dram_tensor("outk", [N_, KPC * CHUNK], F32, kind="ExternalOutput").ap()
    outv = nc.dram_tensor("outv", [N_, KPC * CHUNK], F32, kind="ExternalOutput").ap()
    with tile.TileContext(nc) as tc:
        retrieval_kernel(tc, qT, keys, vals, outk, outv, S_, N_, repeat=repeat)
    nc.compile()
    return nc


_NC = None
LAST_RESULTS = None  # BassKernelResults of the most recent kernel() call


def kernel(queries, mem_keys, mem_vals):
    global _NC, LAST_RESULTS
    if _NC is None:
        _NC = build_nc()

    q = np.asarray(queries, dtype=np.float32).reshape(N, H, Dh)
    in_maps = []
    for h in range(H):
        in_maps.append(
            {
                "qT": np.ascontiguousarray(q[:, h, :].T),
                "keys": np.ascontiguousarray(
                    np.asarray(mem_keys[h], dtype=np.float32).reshape(S, CHUNK)
                ),
                "vals": np.ascontiguousarray(
                    np.asarray(mem_vals[h], dtype=np.float32).reshape(S, CHUNK)
                ),
            }
        )

    res = run_bass_kernel_spmd(nc=_NC, in_maps=in_maps, core_ids=list(range(H)))
    LAST_RESULTS = res

    ks = np.stack(
        [res.results[h]["outk"].reshape(N, K, Dh) for h in range(H)], axis=1
    ).reshape(N * H, K, Dh)
    vs = np.stack(
        [res.results[h]["outv"].reshape(N, K, Dh) for h in range(H)], axis=1
    ).reshape(N * H, K, Dh)
    return np.stack([ks, vs]).astype(np.float32)


def bench_exec_ns(queries, mem_keys, mem_vals, iters=5):
    """Median wall time of the 8-core device execute with inputs pre-placed
    on-device (proxy for HW exec time; excludes host<->device transfers)."""
    global _NC
    if _NC is None:
        _NC = build_nc()
    nc = _NC

    import time

    import jax
    from jax.sharding import Mesh, NamedSharding, PartitionSpec

    from concourse import bass2jax as b2j
    from concourse.bass2jax import _bass_exec_p, partition_id_tensor
    from jax.experimental.shard_map import shard_map

    b2j.install_neuronx_cc_hook()

    q = np.asarray(queries, dtype=np.float32).reshape(N, H, Dh)
    in_maps = []
    for h in range(H):
        in_maps.append(
            {
                "qT": np.ascontiguousarray(q[:, h, :].T),
                "keys": np.ascontiguousarray(
                    np.asarray(mem_keys[h], dtype=np.float32).reshape(S, CHUNK)
                ),
                "vals": np.ascontiguousarray(
                    np.asarray(mem_vals[h], dtype=np.float32).reshape(S, CHUNK)
                ),
            }
        )

    import concourse.mybir as mybir_

    partition_name = nc.partition_id_tensor.name if nc.partition_id_tensor else None
    in_names, out_names, out_avals, zero_outs = [], [], [], []
    for alloc in nc.m.functions[0].allocations:
        if not isinstance(alloc, mybir_.MemoryLocationSet):
            continue
        name = alloc.memorylocations[0].name
        if alloc.kind == "ExternalInput":
            if name != partition_name:
                in_names.append(name)
        elif alloc.kind == "ExternalOutput":
            out_names.append(name)
            shape = tuple(alloc.tensor_shape)
            dtype = mybir_.dt.np(alloc.dtype)
            out_avals.append(jax.core.ShapedArray(shape, dtype))
            zero_outs.append(np.zeros(shape, dtype))
    n_params = len(in_names)
    n_outs = len(out_avals)
    all_in_names = in_names + out_names + ([partition_name] if partition_name else [])
    donate = tuple(range(n_params, n_params + n_outs))

    def _body(*args):
        operands = list(args)
        if partition_name is not None:
            operands.append(partition_id_tensor())
        return tuple(
            _bass_exec_p.bind(
                *operands,
                out_avals=tuple(out_avals),
                in_names=tuple(all_in_names),
                out_names=tuple(out_names),
                lowering_input_output_aliases=(),
                sim_require_finite=True,
                sim_require_nnan=True,
                nc=nc,
            )
        )

    devices = jax.devices()[:H]
    mesh = Mesh(np.asarray(devices), ("core",))
    spec = NamedSharding(mesh, PartitionSpec("core"))
    sharded = jax.jit(
        shard_map(
            _body,
            mesh=mesh,
            in_specs=(PartitionSpec("core"),) * (n_params + n_outs),
            out_specs=(PartitionSpec("core"),) * n_outs,
            check_rep=False,
        ),
        donate_argnums=donate,
        keep_unused=True,
    )
    concat_in = [
        jax.device_put(
            np.concatenate([np.asarray(in_maps[c][nm]) for c in range(H)], axis=0),
            spec,
        )
        for nm in in_names
    ]
    jax.block_until_ready(concat_in)

    times = []
    for i in range(iters + 1):
        zeros = [
            jax.device_put(np.zeros((H * z.shape[0], *z.shape[1:]), z.dtype), spec)
            for z in zero_outs
        ]
        jax.block_until_ready(zeros)
        t0 = time.perf_counter()
        outs = sharded(*concat_in, *zeros)
        jax.block_until_ready(outs)
        t1 = time.perf_counter()
        if i > 0:  # skip compile/warmup iteration
            times.append(t1 - t0)
        del outs
    times.sort()
    return int(times[len(times) // 2] * 1e9)
